# revision 1
# baseline (speedup 1.0000x reference)
"""Trainium2 Bass kernel for causal multi-head attention (B=4,T=1024,C=1024,H=16,D=64).

Sharding: 8 cores = 4 batches x 2 query-row parities (even/odd global rows).
Every core runs the IDENTICAL program; per-core variation (batch slice, row
parity) is carried entirely in the input data (xT slice, xTq gather, causal
masks), so one SPMD module serves all cores with no collectives.

Per-core device program:
  phase 1: qT[h] = Wq_h^T @ xTq, kT[h] = Wk_h^T @ xT (PE, fp32r),
           v = x @ Wv -> vext [keys, 65] per (tblock, head) with a ones column
  phase 2: per head: scoresT[k-block] = kT_blk^T @ qT (keys on partitions),
           exp via ACT (scale=1/sqrt(D)); causal masking = elementwise mul by
           host-supplied 0/1 masks on the two boundary blocks; AV accumulates
           oT[65, 512] = [v|1]^T @ attnT giving both the output and sum-exp;
           normalize via DVE reciprocal + PE rank-1 broadcast. Heads are
           software-pipelined (scores of head h run while AVs of head h-1 and
           normalization of head h-2 complete) to keep the in-order PE fed.
  phase 3: out = concat-heads @ Wo^T + bo (PE, fp32r), DMA out.

DMA: loads are split across both HWDGE rings (SP + ACT) and ordered so the
operands of the first matmuls arrive first.
"""
import sys

sys.path.insert(0, "/opt/trn_rl_repo")
import numpy as np

B, T, C, H, D = 4, 1024, 1024, 16, 64
N_CORES = 8
NCT = C // 128  # 8 contraction tiles
NTT = T // 128  # 8 t/key blocks
NP = H // 2  # 8 head pairs
QR = 512  # query rows per core
# suffix start per key block: q-block j attends key blocks <= 2j+1
STARTS = [0, 0, 128, 128, 256, 256, 384, 384]

_CACHE = {}


def _build():
    import concourse.bacc as bacc
    import concourse.mybir as mybir
    import concourse.tile as tile

    F32 = mybir.dt.float32
    F32R = mybir.dt.float32r
    Exp = mybir.ActivationFunctionType.Exp

    def r(ap):
        return ap.bitcast(F32R)

    nc = bacc.Bacc("TRN2", target_bir_lowering=False, debug=False, num_devices=N_CORES)
    xT_d = nc.declare_dram_parameter("xT", [C, T], F32, isOutput=False)
    xTq_d = nc.declare_dram_parameter("xTq", [C, QR], F32, isOutput=False)
    wq_d = nc.declare_dram_parameter("wq", [C, H * D], F32, isOutput=False)
    wk_d = nc.declare_dram_parameter("wk", [C, H * D], F32, isOutput=False)
    wv_d = nc.declare_dram_parameter("wv", [C, H * D], F32, isOutput=False)
    woT_d = nc.declare_dram_parameter("woT", [H * D, C], F32, isOutput=False)
    bob_d = nc.declare_dram_parameter("bob", [128, C], F32, isOutput=False)
    m0_d = nc.declare_dram_parameter("mask0", [128, 128], F32, isOutput=False)
    m1_d = nc.declare_dram_parameter("mask1", [128, 128], F32, isOutput=False)
    ones_d = nc.declare_dram_parameter("ones", [128, 128], F32, isOutput=False)
    out_d = nc.declare_dram_parameter("out", [QR, C], F32, isOutput=True)

    with tile.TileContext(nc) as tc:
        with tc.tile_pool(name="keep", bufs=1) as keep:
            # persistent tiles
            qT = keep.tile([128, NP, QR], F32)  # 2-head stack on partitions
            kT = keep.tile([128, NP, T], F32)
            vext = keep.tile([128, NTT, H, 65], F32)
            m0 = keep.tile([128, 128], F32)
            m1 = keep.tile([128, 128], F32)
            bob = keep.tile([128, C], F32)
            ones64 = keep.tile([1, 64], F32)
            ones_sb = keep.tile([128, 128], F32)
            nc.sync.dma_start(r(m0[:]), r(m0_d[:]))
            nc.sync.dma_start(r(m1[:]), r(m1_d[:]))
            nc.sync.dma_start(r(ones64[:]), r(ones_d[0:1, 0:64]))
            nc.sync.dma_start(r(ones_sb[:]), r(ones_d[:, :]))
            nc.sync.dma_start(bob[:], bob_d[:])
            nc.vector.tensor_copy(
                r(vext[:, :, :, 64:65]), r(ones_sb[:].rearrange("p (a b) -> p a b", a=NTT)))

            # ---------------- phase 1: projections ----------------
            with (
                tc.tile_pool(name="xp", bufs=1) as xp,
                tc.tile_pool(name="wp", bufs=2) as wp,
                tc.tile_pool(name="ps_wide", bufs=2, space="PSUM") as ps_wide,
                tc.tile_pool(name="ps_q", bufs=2, space="PSUM") as ps_q,
            ):
                xT = xp.tile([128, NCT, T], F32)
                xTq = xp.tile([128, NCT, QR], F32)
                wq = wp.tile([128, NCT, H * D], F32, tag="w")
                wk = wp.tile([128, NCT, H * D], F32, tag="w")
                # ring SP: xTq then xT; ring ACT: wq then wk.  First qT matmul
                # needs (xTq c0, wq c0) which are first in each ring.
                for c in range(NCT):
                    nc.sync.dma_start(r(xTq[:, c, :]), r(xTq_d[c * 128:(c + 1) * 128, :]))
                    nc.scalar.dma_start(r(wq[:, c, :]), r(wq_d[c * 128:(c + 1) * 128, :]))
                for c in range(NCT):
                    nc.sync.dma_start(r(xT[:, c, :]), r(xT_d[c * 128:(c + 1) * 128, :]))
                    nc.scalar.dma_start(r(wk[:, c, :]), r(wk_d[c * 128:(c + 1) * 128, :]))

                # qT: per head pair accumulate over c tiles
                for p in range(NP):
                    psq = ps_q.tile([128, QR], F32)
                    for c in range(NCT):
                        nc.tensor.matmul(
                            psq[:],
                            r(wq[:, c, p * 128:(p + 1) * 128]),
                            r(xTq[:, c, :]),
                            start=(c == 0),
                            stop=(c == NCT - 1),
                        )
                    nc.vector.tensor_copy(r(qT[:, p, :]), psq[:])
                # kT
                for p in range(NP):
                    psk = ps_wide.tile([128, T], F32, tag="wide")
                    for c in range(NCT):
                        lhs = r(wk[:, c, p * 128:(p + 1) * 128])
                        nc.tensor.matmul(psk[:, 0:512], lhs, r(xT[:, c, 0:512]),
                                         start=(c == 0), stop=(c == NCT - 1))
                        nc.tensor.matmul(psk[:, 512:1024], lhs, r(xT[:, c, 512:1024]),
                                         start=(c == 0), stop=(c == NCT - 1))
                    nc.vector.tensor_copy(r(kT[:, p, :]), psk[:])
                # v (natural layout) -> vext; wv reuses a freed w slot
                wv = wp.tile([128, NCT, H * D], F32, tag="w")
                for c in range(NCT):
                    nc.scalar.dma_start(r(wv[:, c, :]), r(wv_d[c * 128:(c + 1) * 128, :]))
                for tt in range(NTT):
                    psv = ps_wide.tile([128, H * D], F32, tag="wide")
                    for c in range(NCT):
                        lhs = r(xT[:, c, tt * 128:(tt + 1) * 128])
                        nc.tensor.matmul(psv[:, 0:512], lhs, r(wv[:, c, 0:512]),
                                         start=(c == 0), stop=(c == NCT - 1))
                        nc.tensor.matmul(psv[:, 512:1024], lhs, r(wv[:, c, 512:1024]),
                                         start=(c == 0), stop=(c == NCT - 1))
                    nc.vector.tensor_copy(r(vext[:, tt, :, 0:64]), psv[:].rearrange("p (h d) -> p h d", h=H))

            # ---------------- phase 2 + 3 ----------------
            with tc.tile_pool(name="keep2", bufs=1) as keep2:
                proj_in = keep2.tile([128, NP, QR], F32)
                woT = keep2.tile([128, NP, C], F32)
                # prefetch Wo^T during attention (SP ring is idle now)
                for p in range(NP):
                    nc.sync.dma_start(r(woT[:, p, :]), r(woT_d[p * 128:(p + 1) * 128, :]))
                with (
                    tc.tile_pool(name="attn", bufs=5) as attnp,
                    tc.tile_pool(name="smalls", bufs=3) as smalls,
                    tc.tile_pool(name="ps_s", bufs=2, space="PSUM") as ps_s,
                    tc.tile_pool(name="ps_o", bufs=2, space="PSUM") as ps_o,
                    tc.tile_pool(name="ps_bc", bufs=2, space="PSUM") as ps_bc,
                ):
                    # software pipeline over heads:
                    #   stage S(h): scores+exp+mask for all 8 key blocks
                    #   stage A(h): AV accumulation (consumes stage S tiles)
                    #   stage N(h): normalize into proj_in
                    tiles = {}  # h -> list of (kb, st, attn tile)
                    oTs = {}  # h -> oT psum

                    def stage_s(h):
                        p, po = h // 2, (h % 2) * 64
                        lst = []
                        for j in range(NTT // 2):  # key-block pair (2j, 2j+1)
                            st = STARTS[2 * j]
                            nm = max(512 - st, 256)
                            sps = ps_s.tile([128, 2, 512], F32, tag="s")
                            for sub in range(2):
                                kb = 2 * j + sub
                                nc.tensor.matmul(
                                    sps[:, sub, 512 - nm:],
                                    r(kT[po:po + 64, p, kb * 128:(kb + 1) * 128]),
                                    r(qT[po:po + 64, p, 512 - nm:]),
                                    start=True,
                                    stop=True,
                                )
                            at = attnp.tile([128, 2, 512], F32, tag="at")
                            # one exp covers both key blocks of the pair
                            nc.scalar.activation(r(at[:, :, st:]), sps[:, :, st:], Exp, scale=0.125)
                            for sub in range(2):
                                msk = m0 if sub == 0 else m1
                                nc.vector.tensor_mul(
                                    r(at[:, sub, j * 128:(j + 1) * 128]),
                                    r(at[:, sub, j * 128:(j + 1) * 128]),
                                    r(msk[:]),
                                )
                            lst.append((j, st, at))
                        tiles[h] = lst

                    def stage_a(h):
                        oT = ps_o.tile([65, QR], F32, tag="o")
                        oTs[h] = oT
                        for j, st, at in tiles.pop(h):
                            for sub in range(2):
                                kb = 2 * j + sub
                                nc.tensor.matmul(
                                    oT[:, st:],
                                    r(vext[:, kb, h, :]),
                                    r(at[:, sub, st:]),
                                    start=(kb == 0),
                                    stop=(kb == NTT - 1),
                                    skip_group_check=True,
                                )

                    def stage_n(h):
                        p, po = h // 2, (h % 2) * 64
                        oT = oTs.pop(h)
                        rec = smalls.tile([1, QR], F32, tag="rec")
                        with nc.allow_low_precision(reason="fp32r relabel of fp32 reciprocal"):
                            nc.vector.reciprocal(r(rec[:]), oT[64:65, :])
                        bc = ps_bc.tile([64, QR], F32, tag="bc")
                        nc.tensor.matmul(bc[:], r(ones64[:]), r(rec[:]), start=True, stop=True)
                        bcs = smalls.tile([64, QR], F32, tag="bcs")
                        nc.vector.tensor_copy(bcs[:], bc[:])
                        nc.vector.tensor_mul(r(proj_in[po:po + 64, p, :]), oT[0:64, :], bcs[:])

                    for h in range(H + 2):
                        if h < H:
                            stage_s(h)
                        if 1 <= h <= H:
                            stage_a(h - 1)
                        if h >= 2:
                            stage_n(h - 2)

                # ---------------- phase 3: output projection ----------------
                with (
                    tc.tile_pool(name="fin", bufs=2) as finp,
                    tc.tile_pool(name="ps_f", bufs=2, space="PSUM") as ps_f,
                ):
                    for m in range(QR // 128):
                        psf = ps_f.tile([128, C], F32)
                        for p in range(NP):
                            lhs = r(proj_in[:, p, m * 128:(m + 1) * 128])
                            nc.tensor.matmul(psf[:, 0:512], lhs, r(woT[:, p, 0:512]),
                                             start=(p == 0), stop=(p == NP - 1))
                            nc.tensor.matmul(psf[:, 512:1024], lhs, r(woT[:, p, 512:1024]),
                                             start=(p == 0), stop=(p == NP - 1))
                        fin = finp.tile([128, C], F32, tag="fin")
                        nc.vector.tensor_add(fin[:], psf[:], bob[:])
                        nc.sync.dma_start(out_d[m * 128:(m + 1) * 128, :], fin[:])

    nc.compile()
    return nc


def get_nc():
    if "nc" not in _CACHE:
        _CACHE["nc"] = _build()
    return _CACHE["nc"]


def make_in_maps(x, Wq, Wk, Wv, Wo, bo):
    x = np.asarray(x, dtype=np.float32)
    wq = np.ascontiguousarray(np.asarray(Wq, np.float32).transpose(1, 0, 2).reshape(C, H * D))
    wk = np.ascontiguousarray(np.asarray(Wk, np.float32).transpose(1, 0, 2).reshape(C, H * D))
    wv = np.ascontiguousarray(np.asarray(Wv, np.float32).transpose(1, 0, 2).reshape(C, H * D))
    woT = np.ascontiguousarray(np.asarray(Wo, np.float32).T)
    bob = np.ascontiguousarray(np.broadcast_to(np.asarray(bo, np.float32), (128, C)))
    ones = np.ones((128, 128), np.float32)
    k_ = np.arange(128)[:, None]
    i_ = np.arange(128)[None, :]
    in_maps = []
    for core in range(N_CORES):
        b, par = core // 2, core % 2
        xT = np.ascontiguousarray(x[b].T)
        xTq = np.ascontiguousarray(xT[:, par::2])
        m0 = (k_ <= 2 * i_ + par).astype(np.float32)
        m1 = (k_ <= 2 * i_ + par - 128).astype(np.float32)
        in_maps.append({
            "xT": xT, "xTq": xTq, "wq": wq, "wk": wk, "wv": wv,
            "woT": woT, "bob": bob, "mask0": m0, "mask1": m1, "ones": ones,
        })
    return in_maps


def kernel(x, Wq, Wk, Wv, Wo, bo):
    from concourse.bass_utils import run_bass_kernel_spmd

    nc = get_nc()
    in_maps = make_in_maps(x, Wq, Wk, Wv, Wo, bo)
    res = run_bass_kernel_spmd(nc, in_maps, list(range(N_CORES)))
    out = np.empty((B, T, C), np.float32)
    for core in range(N_CORES):
        b, par = core // 2, core % 2
        out[b, par::2, :] = res.results[core]["out"]
    return out



# revision 9
# speedup vs baseline: 1.3023x; 1.3023x over previous
"""Trainium2 Bass kernel for causal MHA (B=4,T=1024,C=1024,H=16,D=64).

Sharding: 8 cores = 4 batches x 2 head-halves (8 heads per core, full T).
Each core computes q/k/v projections for its 8 heads, causal attention over
all 1024 queries, and a PARTIAL output projection (its heads' rows of Wo).
The two cores of a batch produce partials that the host sums (+bias) during
output assembly — the "all-reduce" of the tensor-parallel split is free on
host, so the device program needs no collectives and no duplicated k/v work.

Per-core device program:
  phase 1: qT[p] = Wq_p^T @ xT, kT[p] = Wk_p^T @ xT (PE), v = x @ Wv ->
           vext [keys, 65] per (tblock, head) with a ones column.
  phase 2: per head h: scoresT tiles [keys, queries] packed so every PSUM
           element is a valid causal cell; exp via ACT (scale=1/8); diagonal
           128x128 blocks masked by a tril mask on the Pool engine; AV
           accumulates oT[65, 1024] = [v|1]^T @ attnT (row 64 = sum-exp);
           normalize: DVE reciprocal -> gpsimd partition_broadcast -> DVE mul.
           Heads are software-pipelined (S/A/N stages).
  phase 3: partial out = concat-heads @ WoT_half (PE), copy, DMA out.
"""
import sys

sys.path.insert(0, "/opt/trn_rl_repo")
import numpy as np

B, T, C, H, D = 4, 1024, 1024, 16, 64
N_CORES = 8
HH = H // 2  # heads per core
NP = HH // 2  # head pairs per core (partition-stacked)
NCT = C // 128  # contraction tiles
NTT = T // 128  # key blocks

# scores tiling per head: tiles of [128, 1024] PSUM (2 banks / 2 slots of
# 512).  Every slot gets EXACTLY ONE matmul whose output is contained in that
# bank (PE matmul output must not cross a PSUM bank boundary, and each bank
# holds a single start/stop accumulation group).  Slot entries: (kb, q0, ln)
# with tile cols [512*slot : 512*slot + ln) <-> queries [q0 : q0+ln).
SCORE_TILES = [
    [(0, 0, 512), (0, 512, 512)],
    [(1, 128, 512), (1, 640, 384)],
    [(2, 256, 512), (2, 768, 256)],
    [(3, 384, 512), (3, 896, 128)],
    [(4, 512, 512), (5, 640, 384)],
    [(6, 768, 256), (7, 896, 128)],
]
# exp coverage per tile: list of (col0, col1) ranges of valid cells
EXP_RANGES = [
    [(0, 1024)], [(0, 896)], [(0, 768)], [(0, 640)], [(0, 896)],
    [(0, 256), (512, 640)],
]
# diagonal-mask positions: (tile_idx, tile_col) for each kb's diag block,
# derived: kb's diag (q = 128*kb + [0,128)) sits at the slot whose q0 == 128*kb
MASK_POS = [(0, 0), (1, 0), (2, 0), (3, 0), (4, 0), (4, 512), (5, 0), (5, 512)]

_CACHE = {}


def _build():
    import concourse.bacc as bacc
    import concourse.mybir as mybir
    import concourse.tile as tile
    from concourse import library_config

    F32 = mybir.dt.float32
    F32R = mybir.dt.float32r
    Exp = mybir.ActivationFunctionType.Exp

    def r(ap):
        return ap.bitcast(F32R)

    nc = bacc.Bacc("TRN2", target_bir_lowering=False, debug=False, num_devices=N_CORES)
    xT_d = nc.declare_dram_parameter("xT", [C, T], F32, isOutput=False)
    wq_d = nc.declare_dram_parameter("wq", [C, HH * D], F32, isOutput=False)
    wk_d = nc.declare_dram_parameter("wk", [C, HH * D], F32, isOutput=False)
    wv_d = nc.declare_dram_parameter("wv", [C, HH * D], F32, isOutput=False)
    woT_d = nc.declare_dram_parameter("woT", [HH * D, C], F32, isOutput=False)
    mask_d = nc.declare_dram_parameter("mask", [128, 128], F32, isOutput=False)
    ones_d = nc.declare_dram_parameter("ones", [128, 128], F32, isOutput=False)
    out_d = nc.declare_dram_parameter("out", [T, C], F32, isOutput=True)

    def mm(out, lhs, rhs, **kw):
        nc.tensor.matmul(out, r(lhs), r(rhs), **kw)

    with tile.TileContext(nc) as tc:
        with tc.tile_pool(name="keep", bufs=1) as keep:
            qT = keep.tile([128, NP, T], F32)
            kT = keep.tile([128, NP, T], F32)
            vext = keep.tile([128, NTT, HH, 65], F32)
            mask = keep.tile([128, 128], F32)
            ones_sb = keep.tile([128, 128], F32)
            proj_in = keep.tile([128, NP, T], F32)
            woT = keep.tile([128, NP, C], F32)
            nc.sync.dma_start(r(mask[:]), r(mask_d[:]))
            nc.sync.dma_start(r(ones_sb[:]), r(ones_d[:]))
            nc.vector.tensor_copy(
                r(vext[:, :, :, 64:65]),
                r(ones_sb[:, 0:64].rearrange("p (a b) -> p a b", a=NTT)),
            )
            nc.gpsimd.load_library(library_config.proxy)

            # ---------------- phase 1: projections ----------------
            with (
                tc.tile_pool(name="xp", bufs=1) as xp,
                tc.tile_pool(name="ps_qk", bufs=4, space="PSUM") as ps_qk,
            ):
                xT = xp.tile([128, NCT, T], F32)
                wq = xp.tile([128, NCT, HH * D], F32)
                wk = xp.tile([128, NCT, HH * D], F32)
                wv = xp.tile([128, NCT, HH * D], F32)
                for c in range(NCT):
                    nc.sync.dma_start(r(xT[:, c, :]), r(xT_d[c * 128:(c + 1) * 128, :]))
                    nc.scalar.dma_start(r(wq[:, c, :]), r(wq_d[c * 128:(c + 1) * 128, :]))
                    nc.scalar.dma_start(r(wk[:, c, :]), r(wk_d[c * 128:(c + 1) * 128, :]))
                for c in range(NCT):
                    nc.scalar.dma_start(r(wv[:, c, :]), r(wv_d[c * 128:(c + 1) * 128, :]))

                # q & k for pair groups (0,1) then (2,3): 4 live 2-bank psum
                # tiles per group (all 8 banks), interleaved per c-tile so PE
                # work unlocks as DMA arrives.
                for pg in ((0, 1), (2, 3)):
                    ps = {}
                    for p in pg:
                        ps[("q", p)] = ps_qk.tile([128, T], F32, tag="qk", name=f"psq{p}")
                        ps[("k", p)] = ps_qk.tile([128, T], F32, tag="qk", name=f"psk{p}")
                    for c in range(NCT):
                        for p in pg:
                            wqc = wq[:, c, p * 128:(p + 1) * 128]
                            wkc = wk[:, c, p * 128:(p + 1) * 128]
                            for half in range(2):
                                s = slice(half * 512, (half + 1) * 512)
                                mm(ps[("q", p)][:, s], wqc, xT[:, c, s],
                                   start=(c == 0), stop=(c == NCT - 1))
                                mm(ps[("k", p)][:, s], wkc, xT[:, c, s],
                                   start=(c == 0), stop=(c == NCT - 1))
                    for i, p in enumerate(pg):
                        if i == 0:
                            nc.scalar.copy(r(qT[:, p, :]), ps[("q", p)][:])
                            nc.scalar.copy(r(kT[:, p, :]), ps[("k", p)][:])
                        else:
                            nc.vector.tensor_copy(r(qT[:, p, :]), ps[("q", p)][:])
                            nc.vector.tensor_copy(r(kT[:, p, :]), ps[("k", p)][:])

                # v: per key block, accumulate over c
                for tt in range(NTT):
                    psvw = ps_qk.tile([128, T], F32, tag="qk", name=f"psv{tt}")
                    psv = psvw[:, 0:HH * D]
                    for c in range(NCT):
                        mm(psv, xT[:, c, tt * 128:(tt + 1) * 128], wv[:, c, :],
                           start=(c == 0), stop=(c == NCT - 1))
                    src = psv.rearrange("p (h d) -> p h d", h=HH)
                    if tt % 2 == 0:
                        nc.scalar.copy(r(vext[:, tt, :, 0:64]), src)
                    else:
                        nc.vector.tensor_copy(r(vext[:, tt, :, 0:64]), src)

            # ---------------- phase 2: attention (pipelined over heads) ----
            with (
                tc.tile_pool(name="attn", bufs=8) as attnp,
                tc.tile_pool(name="smalls", bufs=3) as smalls,
                tc.tile_pool(name="ps_s", bufs=2, space="PSUM") as ps_s,
                tc.tile_pool(name="ps_o", bufs=2, space="PSUM") as ps_o,
            ):
                # prefetch woT while attention runs (sync ring idle now)
                for p in range(NP):
                    nc.sync.dma_start(r(woT[:, p, :]), r(woT_d[p * 128:(p + 1) * 128, :]))

                ats = {}
                oTs = {}

                def stage_s(h):
                    p, po = h // 2, (h % 2) * 64
                    lst = []
                    for ti, slots in enumerate(SCORE_TILES):
                        sps = ps_s.tile([128, T], F32, tag="s")
                        for si, (kb, q0, ln) in enumerate(slots):
                            col = si * 512
                            mm(sps[:, col:col + ln],
                               kT[po:po + 64, p, kb * 128:(kb + 1) * 128],
                               qT[po:po + 64, p, q0:q0 + ln],
                               start=True, stop=True)
                        at = attnp.tile([128, T], F32, tag="at")
                        for c0, c1 in EXP_RANGES[ti]:
                            nc.scalar.activation(r(at[:, c0:c1]), sps[:, c0:c1],
                                                 Exp, scale=0.125)
                        lst.append(at)
                    for ti, col in MASK_POS:
                        nc.gpsimd.tensor_mul(
                            r(lst[ti][:, col:col + 128]),
                            r(lst[ti][:, col:col + 128]),
                            r(mask[:]),
                        )
                    ats[h] = lst

                def stage_a(h):
                    oT = ps_o.tile([65, T], F32, tag="o")
                    oTs[h] = oT
                    lst = ats.pop(h)
                    for ti, slots in enumerate(SCORE_TILES):
                        at = lst[ti]
                        for si, (kb, q0, ln) in enumerate(slots):
                            col = si * 512
                            # split at the oT bank boundary (q = 512)
                            subs = []
                            if q0 < 512:
                                subs.append((col, q0, min(ln, 512 - q0)))
                                if q0 + ln > 512:
                                    subs.append((col + 512 - q0, 512, q0 + ln - 512))
                            else:
                                subs.append((col, q0, ln))
                            for scol, sq0, sln in subs:
                                last = (kb == 3 and sq0 < 512) or kb == 7
                                mm(oT[:, sq0:sq0 + sln], vext[:, kb, h, :],
                                   at[:, scol:scol + sln],
                                   start=(kb == 0), stop=last,
                                   skip_group_check=True)

                def stage_n(h):
                    p, po = h // 2, (h % 2) * 64
                    oT = oTs.pop(h)
                    rec = smalls.tile([1, T], F32, tag="rec")
                    nc.vector.reciprocal(rec[:], oT[64:65, :])
                    bc = smalls.tile([64, T], F32, tag="bc")
                    nc.gpsimd.partition_broadcast(bc[:], rec[:])
                    nc.vector.tensor_mul(r(proj_in[po:po + 64, p, :]), oT[0:64, :], bc[:])

                for h in range(HH + 2):
                    if h < HH:
                        stage_s(h)
                    if 1 <= h <= HH:
                        stage_a(h - 1)
                    if h >= 2:
                        stage_n(h - 2)

            # ---------------- phase 3: partial output projection ----------
            with (
                tc.tile_pool(name="fin", bufs=2) as finp,
                tc.tile_pool(name="ps_f", bufs=2, space="PSUM") as ps_f,
            ):
                for m in range(T // 128):
                    psf = ps_f.tile([128, C], F32, tag="f")
                    for p in range(NP):
                        lhs = proj_in[:, p, m * 128:(m + 1) * 128]
                        mm(psf[:, 0:512], lhs, woT[:, p, 0:512],
                           start=(p == 0), stop=(p == NP - 1))
                        mm(psf[:, 512:1024], lhs, woT[:, p, 512:1024],
                           start=(p == 0), stop=(p == NP - 1))
                    fin = finp.tile([128, C], F32, tag="fin")
                    if m % 2 == 0:
                        nc.scalar.copy(fin[:], psf[:])
                    else:
                        nc.vector.tensor_copy(fin[:], psf[:])
                    nc.sync.dma_start(out_d[m * 128:(m + 1) * 128, :], fin[:])

    nc.compile()
    return nc


def get_nc():
    if "nc" not in _CACHE:
        _CACHE["nc"] = _build()
    return _CACHE["nc"]


def make_in_maps(x, Wq, Wk, Wv, Wo, bo):
    x = np.asarray(x, dtype=np.float32)
    Wq = np.asarray(Wq, np.float32)
    Wk = np.asarray(Wk, np.float32)
    Wv = np.asarray(Wv, np.float32)
    Wo = np.asarray(Wo, np.float32)
    k_ = np.arange(128)[:, None]
    i_ = np.arange(128)[None, :]
    mask = (k_ <= i_).astype(np.float32)
    ones = np.ones((128, 128), np.float32)
    xTs = [np.ascontiguousarray(x[b].T) for b in range(B)]
    whalf = {}
    for hh in range(2):
        sl = slice(hh * HH, (hh + 1) * HH)
        whalf[hh] = {
            "wq": np.ascontiguousarray(Wq[sl].transpose(1, 0, 2).reshape(C, HH * D)),
            "wk": np.ascontiguousarray(Wk[sl].transpose(1, 0, 2).reshape(C, HH * D)),
            "wv": np.ascontiguousarray(Wv[sl].transpose(1, 0, 2).reshape(C, HH * D)),
            "woT": np.ascontiguousarray(Wo[:, hh * HH * D:(hh + 1) * HH * D].T),
        }
    in_maps = []
    for core in range(N_CORES):
        b, hh = core // 2, core % 2
        m = {"xT": xTs[b], "mask": mask, "ones": ones}
        m.update(whalf[hh])
        in_maps.append(m)
    return in_maps


def kernel(x, Wq, Wk, Wv, Wo, bo):
    from concourse.bass_utils import run_bass_kernel_spmd

    nc = get_nc()
    in_maps = make_in_maps(x, Wq, Wk, Wv, Wo, bo)
    res = run_bass_kernel_spmd(nc, in_maps, list(range(N_CORES)))
    bo = np.asarray(bo, np.float32)
    out = np.empty((B, T, C), np.float32)
    for b in range(B):
        out[b] = res.results[2 * b]["out"] + res.results[2 * b + 1]["out"] + bo
    return out


# revision 23
# speedup vs baseline: 1.5826x; 1.2153x over previous
"""Trainium2 Bass kernel for causal MHA (B=4,T=1024,C=1024,H=16,D=64).

Sharding: 8 cores = 4 batches x 2 head-halves (8 heads per core, full T).
Each core computes q/k/v projections for its 8 heads, causal attention over
all 1024 queries, and a PARTIAL output projection (its heads' rows of Wo).
The two cores of a batch produce partials the host sums (+bias) during
output assembly — the tensor-parallel "all-reduce" is free on host, so the
device program needs no collectives and no duplicated k/v work.

All operands are bf16 (inputs pre-converted on host); PSUM accumulation and
the softmax denominators stay f32.  bf16 halves DMA and SBUF and runs the
PE at 1 cycle/row for every piece size.

Per-core device program:
  phase 1a: qT[p] = Wq_p^T @ xT, kT[p] = Wk_p^T @ xT (PE, 8 PSUM banks).
  phase 1b: v = x @ Wv -> vext [keys, 65] (+ones col), INTERLEAVED with the
            first 3 heads' score stages so the ACT exp pipeline starts early.
  phase 2: per head h: scoresT slot-packed [keys, queries] PSUM tiles (one
           matmul per bank, only valid causal cells); exp via ACT
           (scale=1/8); diagonal 128x128 blocks masked via tril mul on Pool;
           AV accumulates oT[65,1024] f32 = [v|1]^T @ attnT (row 64 =
           sum-exp); normalize: DVE reciprocal -> gpsimd partition_broadcast
           -> DVE mul (casts to bf16).  Heads software-pipelined A/N/S.
  phase 3: partial out = concat-heads @ WoT_half (PE), copy, DMA out (bf16);
           host upcasts, sums core pairs, adds bias.
"""
import sys

sys.path.insert(0, "/opt/trn_rl_repo")
import numpy as np

B, T, C, H, D = 4, 1024, 1024, 16, 64
N_CORES = 8
HH = H // 2  # heads per core
NP = HH // 2  # head pairs per core (partition-stacked)
NCT = C // 128  # contraction tiles
NTT = T // 128  # key blocks
PRELUDE = 3  # heads whose S stage is interleaved with the v projection

# scores tiling per head: tiles of [128, 1024] PSUM (2 banks / 2 slots of
# 512).  Every slot gets EXACTLY ONE matmul whose output is contained in that
# bank (PE matmul output must not cross a PSUM bank boundary, and each bank
# holds a single start/stop accumulation group).  Slot entries: (kb, q0, ln)
# with tile cols [512*slot : 512*slot + ln) <-> queries [q0 : q0+ln).
SCORE_TILES = [
    [(0, 0, 512), (0, 512, 512)],
    [(1, 128, 512), (1, 640, 384)],
    [(2, 256, 512), (2, 768, 256)],
    [(3, 384, 512), (3, 896, 128)],
    [(4, 512, 512), (5, 640, 384)],
    [(6, 768, 256), (7, 896, 128)],
]
# exp coverage per tile: list of (col0, col1) ranges of valid cells
EXP_RANGES = [
    [(0, 1024)], [(0, 896)], [(0, 768)], [(0, 640)], [(0, 896)],
    [(0, 256), (512, 640)],
]
# diagonal-mask positions: (tile_idx, tile_col) for each kb's diag block
MASK_POS = [(0, 0), (1, 0), (2, 0), (3, 0), (4, 0), (4, 512), (5, 0), (5, 512)]

_CACHE = {}


def _build():
    import concourse.bacc as bacc
    import concourse.mybir as mybir
    import concourse.tile as tile
    from concourse import library_config

    F32 = mybir.dt.float32
    BF16 = mybir.dt.bfloat16
    Exp = mybir.ActivationFunctionType.Exp

    nc = bacc.Bacc("TRN2", target_bir_lowering=False, debug=False, num_devices=N_CORES)
    xT_d = nc.declare_dram_parameter("xT", [C, T], BF16, isOutput=False)
    wq_d = nc.declare_dram_parameter("wq", [C, HH * D], BF16, isOutput=False)
    wk_d = nc.declare_dram_parameter("wk", [C, HH * D], BF16, isOutput=False)
    wv_d = nc.declare_dram_parameter("wv", [C, HH * D], BF16, isOutput=False)
    woT_d = nc.declare_dram_parameter("woT", [HH * D, C], BF16, isOutput=False)
    mask_d = nc.declare_dram_parameter("mask", [128, 128], BF16, isOutput=False)
    ones_d = nc.declare_dram_parameter("ones", [128, 128], BF16, isOutput=False)
    out_d = nc.declare_dram_parameter("out", [T, C], BF16, isOutput=True)

    mm = nc.tensor.matmul

    with tile.TileContext(nc) as tc:
        with tc.tile_pool(name="keep", bufs=1) as keep:
            qT = keep.tile([128, NP, T], BF16)
            kT = keep.tile([128, NP, T], BF16)
            vext = keep.tile([128, NTT, HH, 65], BF16)
            mask = keep.tile([128, 128], BF16)
            ones_sb = keep.tile([128, 128], BF16)
            proj_in = keep.tile([128, NP, T], BF16)
            woT = keep.tile([128, NP, C], BF16)
            nc.gpsimd.load_library(library_config.proxy)

            with tc.tile_pool(name="xp", bufs=1) as xp:
                xT = xp.tile([128, NCT, T], BF16)
                wq = xp.tile([128, NCT, HH * D], BF16)
                wk = xp.tile([128, NCT, HH * D], BF16)
                wv = xp.tile([128, NCT, HH * D], BF16)
                # batched transfers (HWDGE generation is ~630ns per DMACopy
                # and serialized, so fewer/bigger transfers win); c0 split so
                # the first matmuls start ASAP.
                def drview(d, c0, c1, cols):
                    return d[c0 * 128:c1 * 128, 0:cols].rearrange(
                        "(c p) t -> p c t", p=128)

                nc.sync.dma_start(xT[:, 0, 0:512], xT_d[0:128, 0:512])
                nc.scalar.dma_start(wq[:, 0, :], wq_d[0:128, :])
                nc.sync.dma_start(xT[:, 0, 512:1024], xT_d[0:128, 512:1024])
                nc.sync.dma_start(xT[:, 1:4, :], drview(xT_d, 1, 4, T))
                nc.scalar.dma_start(wq[:, 1:4, :], drview(wq_d, 1, 4, HH * D))
                nc.scalar.dma_start(wq[:, 4:8, :], drview(wq_d, 4, 8, HH * D))
                nc.sync.dma_start(xT[:, 4:8, :], drview(xT_d, 4, 8, T))
                nc.scalar.dma_start(wk[:, 0:4, :], drview(wk_d, 0, 4, HH * D))
                nc.scalar.dma_start(wk[:, 4:8, :], drview(wk_d, 4, 8, HH * D))
                nc.sync.dma_start(mask[:], mask_d[:])
                nc.sync.dma_start(ones_sb[:], ones_d[:])
                nc.vector.tensor_copy(
                    vext[:, :, :, 64:65],
                    ones_sb[:, 0:64].rearrange("p (a b) -> p a b", a=NTT),
                )
                nc.scalar.dma_start(wv[:, 0:8, :], drview(wv_d, 0, 8, HH * D))
                nc.sync.dma_start(
                    woT[:, :, :],
                    woT_d[:, :].rearrange("(c p) t -> p c t", p=128))

                # ---------- phase 1a: q then k (all 8 PSUM banks) ----------
                with tc.tile_pool(name="ps_qk", bufs=4, space="PSUM") as ps_qk:
                    # PE p-state warmup on a zeroed tile while DMA lands
                    warm = xp.tile([128, 512], BF16)
                    nc.vector.memset(warm[:], 0.0)
                    # preload the Exp activation table while ACT is idle
                    scrap = xp.tile([1, 2], BF16)
                    nc.scalar.activation(scrap[:], warm[0:1, 0:2], Exp, scale=0.125)
                    psw = ps_qk.tile([128, T], F32, tag="qk", name="psw")
                    for i in range(7):
                        mm(psw[:, 0:512], warm[:, 0:128], warm[:],
                           start=True, stop=True)

                    for what, w, dst in (("q", wq, qT), ("k", wk, kT)):
                        ps = {}
                        for p in range(NP):
                            ps[p] = ps_qk.tile([128, T], F32, tag="qk",
                                               name=f"ps{what}{p}")
                        for c in range(NCT):
                            for p in range(NP):
                                wc = w[:, c, p * 128:(p + 1) * 128]
                                for half in range(2):
                                    s = slice(half * 512, (half + 1) * 512)
                                    mm(ps[p][:, s], wc, xT[:, c, s],
                                       start=(c == 0), stop=(c == NCT - 1))
                        for p in range(NP):
                            if p % 2 == 0:
                                nc.scalar.copy(dst[:, p, :], ps[p][:])
                            else:
                                nc.vector.tensor_copy(dst[:, p, :], ps[p][:])

                # ---------- phases 1b + 2 + 3 ----------
                with (
                    tc.tile_pool(name="attn", bufs=2 + 6 * (PRELUDE + 1) + 6) as attnp,
                    tc.tile_pool(name="smalls", bufs=3) as smalls,
                    tc.tile_pool(name="ps_s", bufs=2, space="PSUM") as ps_s,
                ):
                    ats = {}
                    oTs = {}

                    def s_tile(h, ti, lst):
                        p, po = h // 2, (h % 2) * 64
                        slots = SCORE_TILES[ti]
                        sps = ps_s.tile([128, T], F32, tag="s")
                        for si, (kb, q0, ln) in enumerate(slots):
                            col = si * 512
                            mm(sps[:, col:col + ln],
                               kT[po:po + 64, p, kb * 128:(kb + 1) * 128],
                               qT[po:po + 64, p, q0:q0 + ln],
                               start=True, stop=True)
                        at = attnp.tile([128, T], BF16, tag="at")
                        for c0, c1 in EXP_RANGES[ti]:
                            nc.scalar.activation(at[:, c0:c1], sps[:, c0:c1],
                                                 Exp, scale=0.125)
                        for mti, col in MASK_POS:
                            if mti == ti:
                                nc.gpsimd.tensor_mul(
                                    at[:, col:col + 128],
                                    at[:, col:col + 128],
                                    mask[:],
                                )
                        lst.append(at)

                    def stage_s(h):
                        lst = []
                        for ti in range(len(SCORE_TILES)):
                            s_tile(h, ti, lst)
                        ats[h] = lst

                    def a_tile(h, ti, oT, lst):
                        at = lst[ti]
                        for si, (kb, q0, ln) in enumerate(SCORE_TILES[ti]):
                            col = si * 512
                            subs = []
                            if q0 < 512:
                                subs.append((col, q0, min(ln, 512 - q0)))
                                if q0 + ln > 512:
                                    subs.append((col + 512 - q0, 512,
                                                 q0 + ln - 512))
                            else:
                                subs.append((col, q0, ln))
                            for scol, sq0, sln in subs:
                                last = (kb == 3 and sq0 < 512) or kb == 7
                                mm(oT[:, sq0:sq0 + sln], vext[:, kb, h, :],
                                   at[:, scol:scol + sln],
                                   start=(kb == 0), stop=last,
                                   skip_group_check=True)

                    def stage_n(h):
                        p, po = h // 2, (h % 2) * 64
                        oT = oTs.pop(h)
                        rec = smalls.tile([1, T], F32, tag="rec")
                        nc.vector.reciprocal(rec[:], oT[64:65, :])
                        bc = smalls.tile([64, T], F32, tag="bc")
                        nc.gpsimd.partition_broadcast(bc[:], rec[:])
                        nc.vector.tensor_mul(proj_in[po:po + 64, p, :],
                                             oT[0:64, :], bc[:])

                    # -------- phase 1b: v interleaved with S(0..PRELUDE-1) --
                    with tc.tile_pool(name="ps_v", bufs=2, space="PSUM") as ps_v:
                        vjobs = []
                        for tt in range(NTT):
                            vjobs.append(("alloc", tt))
                            for c in range(NCT):
                                vjobs.append(("mm", tt, c))
                            vjobs.append(("copy", tt))
                        sjobs = [("s", h) for h in range(PRELUDE)]
                        # interleave: ~3 v-jobs per score stage
                        psvs = {}
                        vi = si_ = 0
                        while vi < len(vjobs) or si_ < len(sjobs):
                            for _ in range(30):
                                if vi >= len(vjobs):
                                    break
                                job = vjobs[vi]
                                vi += 1
                                if job[0] == "alloc":
                                    tt = job[1]
                                    psvw = ps_v.tile([128, 512], F32, tag="v",
                                                     name=f"psv{tt}")
                                    psvs[tt] = psvw
                                elif job[0] == "mm":
                                    _, tt, c = job
                                    mm(psvs[tt][:],
                                       xT[:, c, tt * 128:(tt + 1) * 128],
                                       wv[:, c, :],
                                       start=(c == 0), stop=(c == NCT - 1))
                                else:
                                    tt = job[1]
                                    src = psvs.pop(tt)[:].rearrange(
                                        "p (h d) -> p h d", h=HH)
                                    if tt % 2 == 0:
                                        nc.scalar.copy(vext[:, tt, :, 0:64], src)
                                    else:
                                        nc.vector.tensor_copy(
                                            vext[:, tt, :, 0:64], src)
                            if si_ < len(sjobs):
                                stage_s(sjobs[si_][1])
                                si_ += 1

                    # -------- phase 2 main: pipeline A/N/S over heads, with
                    # S(h+PRELUDE) and A(h) interleaved at tile granularity so
                    # ACT always has score input queued just ahead of the AVs.
                    with tc.tile_pool(name="ps_o", bufs=2, space="PSUM") as ps_o:
                        for h in range(HH):
                            if h >= 1:
                                stage_n(h - 1)
                            oT = ps_o.tile([65, T], F32, tag="o")
                            oTs[h] = oT
                            hs = h + PRELUDE
                            slst = [] if hs < HH else None
                            alst = ats.pop(h)
                            for ti in range(len(SCORE_TILES)):
                                if slst is not None:
                                    s_tile(hs, ti, slst)
                                a_tile(h, ti, oT, alst)
                            if slst is not None:
                                ats[hs] = slst
                        stage_n(HH - 1)

            # ---------------- phase 3: partial output projection ----------
            with (
                tc.tile_pool(name="fin", bufs=4) as finp,
                tc.tile_pool(name="ps_f", bufs=2, space="PSUM") as ps_f,
            ):
                for m in range(T // 128):
                    psf = ps_f.tile([128, C], F32, tag="f")
                    for p in range(NP):
                        lhs = proj_in[:, p, m * 128:(m + 1) * 128]
                        mm(psf[:, 0:512], lhs, woT[:, p, 0:512],
                           start=(p == 0), stop=(p == NP - 1))
                        mm(psf[:, 512:1024], lhs, woT[:, p, 512:1024],
                           start=(p == 0), stop=(p == NP - 1))
                    fin = finp.tile([128, C], BF16, tag="fin")
                    eng = (nc.scalar, nc.vector, nc.gpsimd)[m % 3]
                    if eng is nc.scalar:
                        nc.scalar.copy(fin[:], psf[:])
                    else:
                        eng.tensor_copy(fin[:], psf[:])
                    ring = nc.sync if m % 2 == 0 else nc.scalar
                    ring.dma_start(out_d[m * 128:(m + 1) * 128, :], fin[:])

    nc.compile()
    return nc


def get_nc():
    if "nc" not in _CACHE:
        _CACHE["nc"] = _build()
    return _CACHE["nc"]


def make_in_maps(x, Wq, Wk, Wv, Wo, bo):
    import ml_dtypes

    bf16 = ml_dtypes.bfloat16
    x = np.asarray(x, dtype=np.float32)
    Wq = np.asarray(Wq, np.float32)
    Wk = np.asarray(Wk, np.float32)
    Wv = np.asarray(Wv, np.float32)
    Wo = np.asarray(Wo, np.float32)
    k_ = np.arange(128)[:, None]
    i_ = np.arange(128)[None, :]
    mask = (k_ <= i_).astype(bf16)
    ones = np.ones((128, 128), bf16)
    xTs = [np.ascontiguousarray(x[b].T).astype(bf16) for b in range(B)]
    whalf = {}
    for hh in range(2):
        sl = slice(hh * HH, (hh + 1) * HH)
        whalf[hh] = {
            "wq": Wq[sl].transpose(1, 0, 2).reshape(C, HH * D).astype(bf16),
            "wk": Wk[sl].transpose(1, 0, 2).reshape(C, HH * D).astype(bf16),
            "wv": Wv[sl].transpose(1, 0, 2).reshape(C, HH * D).astype(bf16),
            "woT": np.ascontiguousarray(
                Wo[:, hh * HH * D:(hh + 1) * HH * D].T).astype(bf16),
        }
    in_maps = []
    for core in range(N_CORES):
        b, hh = core // 2, core % 2
        m = {"xT": xTs[b], "mask": mask, "ones": ones}
        m.update(whalf[hh])
        in_maps.append(m)
    return in_maps


def kernel(x, Wq, Wk, Wv, Wo, bo):
    from concourse.bass_utils import run_bass_kernel_spmd

    nc = get_nc()
    in_maps = make_in_maps(x, Wq, Wk, Wv, Wo, bo)
    res = run_bass_kernel_spmd(nc, in_maps, list(range(N_CORES)))
    _CACHE["last_result"] = res
    bo = np.asarray(bo, np.float32)
    out = np.empty((B, T, C), np.float32)
    for b in range(B):
        out[b] = (res.results[2 * b]["out"].astype(np.float32)
                  + res.results[2 * b + 1]["out"].astype(np.float32) + bo)
    return out


# revision 24
# speedup vs baseline: 1.5839x; 1.0008x over previous
"""Trainium2 Bass kernel for causal MHA (B=4,T=1024,C=1024,H=16,D=64).

Sharding: 8 cores = 4 batches x 2 head-halves (8 heads per core, full T).
Each core computes q/k/v projections for its 8 heads, causal attention over
all 1024 queries, and a PARTIAL output projection (its heads' rows of Wo).
The two cores of a batch produce partials the host sums (+bias) during
output assembly — the tensor-parallel "all-reduce" is free on host, so the
device program needs no collectives and no duplicated k/v work.

All operands are bf16 (inputs pre-converted on host); PSUM accumulation and
the softmax denominators stay f32.  bf16 halves DMA and SBUF and runs the
PE at 1 cycle/row for every piece size.

Per-core device program:
  phase 1a: qT[p] = Wq_p^T @ xT, kT[p] = Wk_p^T @ xT (PE, 8 PSUM banks).
  phase 1b: v = x @ Wv -> vext [keys, 65] (+ones col), INTERLEAVED with the
            first 3 heads' score stages so the ACT exp pipeline starts early.
  phase 2: per head h: scoresT slot-packed [keys, queries] PSUM tiles (one
           matmul per bank, only valid causal cells); exp via ACT
           (scale=1/8); diagonal 128x128 blocks masked via tril mul on Pool;
           AV accumulates oT[65,1024] f32 = [v|1]^T @ attnT (row 64 =
           sum-exp); normalize: DVE reciprocal -> gpsimd partition_broadcast
           -> DVE mul (casts to bf16).  Heads software-pipelined A/N/S.
  phase 3: partial out = concat-heads @ WoT_half (PE), copy, DMA out (bf16);
           host upcasts, sums core pairs, adds bias.
"""
import sys

sys.path.insert(0, "/opt/trn_rl_repo")
import numpy as np

B, T, C, H, D = 4, 1024, 1024, 16, 64
N_CORES = 8
HH = H // 2  # heads per core
NP = HH // 2  # head pairs per core (partition-stacked)
NCT = C // 128  # contraction tiles
NTT = T // 128  # key blocks
PRELUDE = 3  # heads whose S stage is interleaved with the v projection

# scores tiling per head: tiles of [128, 1024] PSUM (2 banks / 2 slots of
# 512).  Every slot gets EXACTLY ONE matmul whose output is contained in that
# bank (PE matmul output must not cross a PSUM bank boundary, and each bank
# holds a single start/stop accumulation group).  Slot entries: (kb, q0, ln)
# with tile cols [512*slot : 512*slot + ln) <-> queries [q0 : q0+ln).
SCORE_TILES = [
    [(0, 0, 512), (0, 512, 512)],
    [(1, 128, 512), (1, 640, 384)],
    [(2, 256, 512), (2, 768, 256)],
    [(3, 384, 512), (3, 896, 128)],
    [(4, 512, 512), (5, 640, 384)],
    [(6, 768, 256), (7, 896, 128)],
]
# exp coverage per tile: list of (col0, col1) ranges of valid cells
EXP_RANGES = [
    [(0, 1024)], [(0, 896)], [(0, 768)], [(0, 640)], [(0, 896)],
    [(0, 256), (512, 640)],
]
# diagonal-mask positions: (tile_idx, tile_col) for each kb's diag block
MASK_POS = [(0, 0), (1, 0), (2, 0), (3, 0), (4, 0), (4, 512), (5, 0), (5, 512)]

_CACHE = {}


def _build():
    import concourse.bacc as bacc
    import concourse.mybir as mybir
    import concourse.tile as tile
    from concourse import library_config

    F32 = mybir.dt.float32
    BF16 = mybir.dt.bfloat16
    Exp = mybir.ActivationFunctionType.Exp

    nc = bacc.Bacc("TRN2", target_bir_lowering=False, debug=False, num_devices=N_CORES)
    xT_d = nc.declare_dram_parameter("xT", [C, T], BF16, isOutput=False)
    wq_d = nc.declare_dram_parameter("wq", [C, HH * D], BF16, isOutput=False)
    wk_d = nc.declare_dram_parameter("wk", [C, HH * D], BF16, isOutput=False)
    wv_d = nc.declare_dram_parameter("wv", [C, HH * D], BF16, isOutput=False)
    woT_d = nc.declare_dram_parameter("woT", [HH * D, C], BF16, isOutput=False)
    mask_d = nc.declare_dram_parameter("mask", [128, 128], BF16, isOutput=False)
    ones_d = nc.declare_dram_parameter("ones", [128, 128], BF16, isOutput=False)
    out_d = nc.declare_dram_parameter("out", [T, C], BF16, isOutput=True)

    mm = nc.tensor.matmul

    with tile.TileContext(nc) as tc:
        with tc.tile_pool(name="keep", bufs=1) as keep:
            qT = keep.tile([128, NP, T], BF16)
            kT = keep.tile([128, NP, T], BF16)
            vext = keep.tile([128, NTT, HH, 65], BF16)
            mask = keep.tile([128, 128], BF16)
            ones_sb = keep.tile([128, 128], BF16)
            proj_in = keep.tile([128, NP, T], BF16)
            woT = keep.tile([128, NP, C], BF16)
            nc.gpsimd.load_library(library_config.proxy)

            with tc.tile_pool(name="xp", bufs=1) as xp:
                xT = xp.tile([128, NCT, T], BF16)
                wq = xp.tile([128, NCT, HH * D], BF16)
                wk = xp.tile([128, NCT, HH * D], BF16)
                wv = xp.tile([128, NCT, HH * D], BF16)
                # batched transfers (HWDGE generation is ~630ns per DMACopy
                # and serialized, so fewer/bigger transfers win); c0 split so
                # the first matmuls start ASAP.
                def drview(d, c0, c1, cols):
                    return d[c0 * 128:c1 * 128, 0:cols].rearrange(
                        "(c p) t -> p c t", p=128)

                nc.sync.dma_start(xT[:, 0, 0:512], xT_d[0:128, 0:512])
                nc.scalar.dma_start(wq[:, 0, :], wq_d[0:128, :])
                nc.sync.dma_start(xT[:, 0, 512:1024], xT_d[0:128, 512:1024])
                nc.sync.dma_start(xT[:, 1:4, :], drview(xT_d, 1, 4, T))
                nc.scalar.dma_start(wq[:, 1:4, :], drview(wq_d, 1, 4, HH * D))
                nc.scalar.dma_start(wq[:, 4:8, :], drview(wq_d, 4, 8, HH * D))
                nc.sync.dma_start(xT[:, 4:8, :], drview(xT_d, 4, 8, T))
                nc.scalar.dma_start(wk[:, 0:4, :], drview(wk_d, 0, 4, HH * D))
                nc.scalar.dma_start(wk[:, 4:8, :], drview(wk_d, 4, 8, HH * D))
                nc.sync.dma_start(mask[:], mask_d[:])
                nc.sync.dma_start(ones_sb[:], ones_d[:])
                nc.vector.tensor_copy(
                    vext[:, :, :, 64:65],
                    ones_sb[:, 0:64].rearrange("p (a b) -> p a b", a=NTT),
                )
                nc.scalar.dma_start(wv[:, 0:8, :], drview(wv_d, 0, 8, HH * D))
                nc.sync.dma_start(
                    woT[:, :, :],
                    woT_d[:, :].rearrange("(c p) t -> p c t", p=128))

                # ---------- phase 1a: q then k (all 8 PSUM banks) ----------
                with tc.tile_pool(name="ps_qk", bufs=4, space="PSUM") as ps_qk:
                    # PE p-state warmup on a zeroed tile while DMA lands
                    warm = xp.tile([128, 512], BF16)
                    nc.vector.memset(warm[:], 0.0)
                    # preload the Exp activation table while ACT is idle
                    scrap = xp.tile([1, 2], BF16)
                    nc.scalar.activation(scrap[:], warm[0:1, 0:2], Exp, scale=0.125)
                    psw = ps_qk.tile([128, T], F32, tag="qk", name="psw")
                    for i in range(7):
                        mm(psw[:, 0:512], warm[:, 0:128], warm[:],
                           start=True, stop=True)

                    for what, w, dst in (("q", wq, qT), ("k", wk, kT)):
                        ps = {}
                        for p in range(NP):
                            ps[p] = ps_qk.tile([128, T], F32, tag="qk",
                                               name=f"ps{what}{p}")
                        for c in range(NCT):
                            for p in range(NP):
                                wc = w[:, c, p * 128:(p + 1) * 128]
                                for half in range(2):
                                    s = slice(half * 512, (half + 1) * 512)
                                    mm(ps[p][:, s], wc, xT[:, c, s],
                                       start=(c == 0), stop=(c == NCT - 1))
                        for p in range(NP):
                            if p % 2 == 0:
                                nc.scalar.copy(dst[:, p, :], ps[p][:])
                            else:
                                nc.vector.tensor_copy(dst[:, p, :], ps[p][:])

                # ---------- phases 1b + 2 + 3 ----------
                with (
                    tc.tile_pool(name="attn", bufs=2 + 6 * (PRELUDE + 1) + 6) as attnp,
                    tc.tile_pool(name="smalls", bufs=3) as smalls,
                    tc.tile_pool(name="ps_s", bufs=2, space="PSUM") as ps_s,
                ):
                    ats = {}
                    oTs = {}

                    def s_tile(h, ti, lst):
                        p, po = h // 2, (h % 2) * 64
                        slots = SCORE_TILES[ti]
                        sps = ps_s.tile([128, T], F32, tag="s")
                        for si, (kb, q0, ln) in enumerate(slots):
                            col = si * 512
                            mm(sps[:, col:col + ln],
                               kT[po:po + 64, p, kb * 128:(kb + 1) * 128],
                               qT[po:po + 64, p, q0:q0 + ln],
                               start=True, stop=True)
                        at = attnp.tile([128, T], BF16, tag="at")
                        for c0, c1 in EXP_RANGES[ti]:
                            nc.scalar.activation(at[:, c0:c1], sps[:, c0:c1],
                                                 Exp, scale=0.125)
                        for mti, col in MASK_POS:
                            if mti == ti:
                                nc.gpsimd.tensor_mul(
                                    at[:, col:col + 128],
                                    at[:, col:col + 128],
                                    mask[:],
                                )
                        lst.append(at)

                    def stage_s(h):
                        lst = []
                        for ti in range(len(SCORE_TILES)):
                            s_tile(h, ti, lst)
                        ats[h] = lst

                    def a_tile(h, ti, oT, lst):
                        at = lst[ti]
                        for si, (kb, q0, ln) in enumerate(SCORE_TILES[ti]):
                            col = si * 512
                            subs = []
                            if q0 < 512:
                                subs.append((col, q0, min(ln, 512 - q0)))
                                if q0 + ln > 512:
                                    subs.append((col + 512 - q0, 512,
                                                 q0 + ln - 512))
                            else:
                                subs.append((col, q0, ln))
                            for scol, sq0, sln in subs:
                                last = (kb == 3 and sq0 < 512) or kb == 7
                                mm(oT[:, sq0:sq0 + sln], vext[:, kb, h, :],
                                   at[:, scol:scol + sln],
                                   start=(kb == 0), stop=last,
                                   skip_group_check=True)

                    def stage_n(h):
                        p, po = h // 2, (h % 2) * 64
                        oT = oTs.pop(h)
                        rec = smalls.tile([1, T], F32, tag="rec")
                        nc.vector.reciprocal(rec[:], oT[64:65, :])
                        bc = smalls.tile([64, T], F32, tag="bc")
                        nc.gpsimd.partition_broadcast(bc[:], rec[:])
                        nc.vector.tensor_mul(proj_in[po:po + 64, p, :],
                                             oT[0:64, :], bc[:])

                    # -------- phase 1b: v interleaved with S(0..PRELUDE-1) --
                    with tc.tile_pool(name="ps_v", bufs=2, space="PSUM") as ps_v:
                        vjobs = []
                        for tt in range(NTT):
                            vjobs.append(("alloc", tt))
                            for c in range(NCT):
                                vjobs.append(("mm", tt, c))
                            vjobs.append(("copy", tt))
                        sjobs = [("s", h) for h in range(PRELUDE)]
                        # interleave: ~3 v-jobs per score stage
                        psvs = {}
                        vi = si_ = 0
                        while vi < len(vjobs) or si_ < len(sjobs):
                            for _ in range(30):
                                if vi >= len(vjobs):
                                    break
                                job = vjobs[vi]
                                vi += 1
                                if job[0] == "alloc":
                                    tt = job[1]
                                    psvw = ps_v.tile([128, 512], F32, tag="v",
                                                     name=f"psv{tt}")
                                    psvs[tt] = psvw
                                elif job[0] == "mm":
                                    _, tt, c = job
                                    mm(psvs[tt][:],
                                       xT[:, c, tt * 128:(tt + 1) * 128],
                                       wv[:, c, :],
                                       start=(c == 0), stop=(c == NCT - 1))
                                else:
                                    tt = job[1]
                                    src = psvs.pop(tt)[:].rearrange(
                                        "p (h d) -> p h d", h=HH)
                                    if tt % 2 == 0:
                                        nc.scalar.copy(vext[:, tt, :, 0:64], src)
                                    else:
                                        nc.vector.tensor_copy(
                                            vext[:, tt, :, 0:64], src)
                            if si_ < len(sjobs):
                                stage_s(sjobs[si_][1])
                                si_ += 1

                    # -------- phase 2 main: pipeline A/N/S over heads, with
                    # S(h+PRELUDE) and A(h) interleaved at tile granularity so
                    # ACT always has score input queued just ahead of the AVs.
                    with tc.tile_pool(name="ps_o", bufs=2, space="PSUM") as ps_o:
                        for h in range(HH):
                            if h >= 1:
                                stage_n(h - 1)
                            oT = ps_o.tile([65, T], F32, tag="o")
                            oTs[h] = oT
                            hs = h + PRELUDE
                            slst = [] if hs < HH else None
                            alst = ats.pop(h)
                            for ti in range(len(SCORE_TILES)):
                                if slst is not None:
                                    s_tile(hs, ti, slst)
                                a_tile(h, ti, oT, alst)
                            if slst is not None:
                                ats[hs] = slst
                        stage_n(HH - 1)

            # ---------------- phase 3: partial output projection ----------
            with (
                tc.tile_pool(name="fin", bufs=4) as finp,
                tc.tile_pool(name="ps_f", bufs=2, space="PSUM") as ps_f,
            ):
                for m in range(T // 128):
                    psf = ps_f.tile([128, C], F32, tag="f")
                    for p in range(NP):
                        lhs = proj_in[:, p, m * 128:(m + 1) * 128]
                        mm(psf[:, 0:512], lhs, woT[:, p, 0:512],
                           start=(p == 0), stop=(p == NP - 1))
                        mm(psf[:, 512:1024], lhs, woT[:, p, 512:1024],
                           start=(p == 0), stop=(p == NP - 1))
                    fin = finp.tile([128, C], BF16, tag="fin")
                    if m % 2 == 0:
                        nc.scalar.copy(fin[:], psf[:])
                    else:
                        nc.vector.tensor_copy(fin[:], psf[:])
                    ring = nc.sync if m % 2 == 0 else nc.scalar
                    ring.dma_start(out_d[m * 128:(m + 1) * 128, :], fin[:])

    nc.compile()
    return nc


def get_nc():
    if "nc" not in _CACHE:
        _CACHE["nc"] = _build()
    return _CACHE["nc"]


def make_in_maps(x, Wq, Wk, Wv, Wo, bo):
    import ml_dtypes

    bf16 = ml_dtypes.bfloat16
    x = np.asarray(x, dtype=np.float32)
    Wq = np.asarray(Wq, np.float32)
    Wk = np.asarray(Wk, np.float32)
    Wv = np.asarray(Wv, np.float32)
    Wo = np.asarray(Wo, np.float32)
    k_ = np.arange(128)[:, None]
    i_ = np.arange(128)[None, :]
    mask = (k_ <= i_).astype(bf16)
    ones = np.ones((128, 128), bf16)
    xTs = [np.ascontiguousarray(x[b].T).astype(bf16) for b in range(B)]
    whalf = {}
    for hh in range(2):
        sl = slice(hh * HH, (hh + 1) * HH)
        whalf[hh] = {
            "wq": Wq[sl].transpose(1, 0, 2).reshape(C, HH * D).astype(bf16),
            "wk": Wk[sl].transpose(1, 0, 2).reshape(C, HH * D).astype(bf16),
            "wv": Wv[sl].transpose(1, 0, 2).reshape(C, HH * D).astype(bf16),
            "woT": np.ascontiguousarray(
                Wo[:, hh * HH * D:(hh + 1) * HH * D].T).astype(bf16),
        }
    in_maps = []
    for core in range(N_CORES):
        b, hh = core // 2, core % 2
        m = {"xT": xTs[b], "mask": mask, "ones": ones}
        m.update(whalf[hh])
        in_maps.append(m)
    return in_maps


def kernel(x, Wq, Wk, Wv, Wo, bo):
    from concourse.bass_utils import run_bass_kernel_spmd

    nc = get_nc()
    in_maps = make_in_maps(x, Wq, Wk, Wv, Wo, bo)
    res = run_bass_kernel_spmd(nc, in_maps, list(range(N_CORES)))
    _CACHE["last_result"] = res
    bo = np.asarray(bo, np.float32)
    out = np.empty((B, T, C), np.float32)
    for b in range(B):
        out[b] = (res.results[2 * b]["out"].astype(np.float32)
                  + res.results[2 * b + 1]["out"].astype(np.float32) + bo)
    return out


# revision 29
# speedup vs baseline: 1.6657x; 1.0517x over previous
"""Trainium2 Bass kernel for causal MHA (B=4,T=1024,C=1024,H=16,D=64).

Sharding: 8 cores = 4 batches x 2 head-halves (8 heads per core, full T).
Each core computes q/k/v projections for its 8 heads, causal attention over
all 1024 queries, and a PARTIAL output projection (its heads' rows of Wo).
The two cores of a batch produce partials the host sums (+bias) during
output assembly — the tensor-parallel "all-reduce" is free on host, so the
device program needs no collectives and no duplicated k/v work.

All operands are bf16 (inputs pre-converted on host); PSUM accumulation and
the softmax denominators stay f32.  bf16 halves DMA and SBUF and runs the
PE at 1 cycle/row for every piece size.

Per-core device program:
  phase 1a: qT[p] = Wq_p^T @ xT, kT[p] = Wk_p^T @ xT (PE, 8 PSUM banks).
  phase 1b: v = x @ Wv -> vext [keys, 65] (+ones col), INTERLEAVED with the
            first 3 heads' score stages so the ACT exp pipeline starts early.
  phase 2: per head h: scoresT slot-packed [keys, queries] PSUM tiles (one
           matmul per bank, only valid causal cells); exp via ACT
           (scale=1/8); diagonal 128x128 blocks masked via tril mul on Pool;
           AV accumulates oT[65,1024] f32 = [v|1]^T @ attnT (row 64 =
           sum-exp); normalize: DVE reciprocal -> gpsimd partition_broadcast
           -> DVE mul (casts to bf16).  Heads software-pipelined A/N/S.
  phase 3: partial out = concat-heads @ WoT_half (PE), copy, DMA out (bf16);
           host upcasts, sums core pairs, adds bias.
"""
import sys

sys.path.insert(0, "/opt/trn_rl_repo")
import numpy as np

B, T, C, H, D = 4, 1024, 1024, 16, 64
N_CORES = 8
HH = H // 2  # heads per core
NP = HH // 2  # head pairs per core (partition-stacked)
NCT = C // 128  # contraction tiles
NTT = T // 128  # key blocks
PRELUDE = 3  # heads whose S stage is interleaved with the v projection

# scores tiling per head: tiles of [128, 1024] PSUM (2 banks / 2 slots of
# 512).  Every slot gets EXACTLY ONE matmul whose output is contained in that
# bank (PE matmul output must not cross a PSUM bank boundary, and each bank
# holds a single start/stop accumulation group).  Slot entries: (kb, q0, ln)
# with tile cols [512*slot : 512*slot + ln) <-> queries [q0 : q0+ln).
SCORE_TILES = [
    [(0, 0, 512), (0, 512, 512)],
    [(1, 128, 512), (1, 640, 384)],
    [(2, 256, 512), (2, 768, 256)],
    [(3, 384, 512), (3, 896, 128)],
    [(4, 512, 512), (5, 640, 384)],
    [(6, 768, 256), (7, 896, 128)],
]
# exp coverage per tile: list of (col0, col1) ranges of valid cells
EXP_RANGES = [
    [(0, 1024)], [(0, 896)], [(0, 768)], [(0, 640)], [(0, 896)],
    [(0, 256), (512, 640)],
]
# diagonal-mask positions: (tile_idx, tile_col) for each kb's diag block
MASK_POS = [(0, 0), (1, 0), (2, 0), (3, 0), (4, 0), (4, 512), (5, 0), (5, 512)]

_CACHE = {}


def _build():
    import concourse.bacc as bacc
    import concourse.mybir as mybir
    import concourse.tile as tile
    from concourse import library_config

    F32 = mybir.dt.float32
    BF16 = mybir.dt.bfloat16
    Exp = mybir.ActivationFunctionType.Exp

    nc = bacc.Bacc("TRN2", target_bir_lowering=False, debug=False, num_devices=N_CORES)
    xT_d = nc.declare_dram_parameter("xT", [C, T], BF16, isOutput=False)
    wq_d = nc.declare_dram_parameter("wq", [C, HH * D], BF16, isOutput=False)
    wk_d = nc.declare_dram_parameter("wk", [C, HH * D], BF16, isOutput=False)
    wv_d = nc.declare_dram_parameter("wv", [C, HH * D], BF16, isOutput=False)
    woT_d = nc.declare_dram_parameter("woT", [HH * D, C], BF16, isOutput=False)
    mask_d = nc.declare_dram_parameter("mask", [128, 128], BF16, isOutput=False)
    ones_d = nc.declare_dram_parameter("ones", [128, 128], BF16, isOutput=False)
    out_d = nc.declare_dram_parameter("out", [T, C], BF16, isOutput=True)

    mm = nc.tensor.matmul

    with tile.TileContext(nc) as tc:
        with tc.tile_pool(name="keep", bufs=1) as keep:
            qT = keep.tile([128, NP, T], BF16)
            kT = keep.tile([128, NP, T], BF16)
            vext = keep.tile([128, NTT, HH, 65], BF16)
            mask = keep.tile([128, 128], BF16)
            ones_sb = keep.tile([128, 128], BF16)
            proj_in = keep.tile([128, NP, T], BF16)
            woT = keep.tile([128, NP, C], BF16)
            nc.gpsimd.load_library(library_config.proxy)

            with tc.tile_pool(name="xp", bufs=1) as xp:
                xT = xp.tile([128, NCT, T], BF16)
                wq = xp.tile([128, NCT, HH * D], BF16)
                wk = xp.tile([128, NCT, HH * D], BF16)
                wv = xp.tile([128, NCT, HH * D], BF16)
                # batched transfers (HWDGE generation is ~630ns per DMACopy
                # and serialized, so fewer/bigger transfers win); c0 split so
                # the first matmuls start ASAP.
                def drview(d, c0, c1, cols):
                    return d[c0 * 128:c1 * 128, 0:cols].rearrange(
                        "(c p) t -> p c t", p=128)

                nc.sync.dma_start(xT[:, 0, 0:512], xT_d[0:128, 0:512])
                nc.scalar.dma_start(wq[:, 0, :], wq_d[0:128, :])
                nc.sync.dma_start(xT[:, 0, 512:1024], xT_d[0:128, 512:1024])
                nc.sync.dma_start(xT[:, 1:4, :], drview(xT_d, 1, 4, T))
                nc.scalar.dma_start(wq[:, 1:4, :], drview(wq_d, 1, 4, HH * D))
                nc.scalar.dma_start(wq[:, 4:8, :], drview(wq_d, 4, 8, HH * D))
                nc.sync.dma_start(xT[:, 4:8, :], drview(xT_d, 4, 8, T))
                nc.scalar.dma_start(wk[:, 0:4, :], drview(wk_d, 0, 4, HH * D))
                nc.scalar.dma_start(wk[:, 4:8, :], drview(wk_d, 4, 8, HH * D))
                nc.sync.dma_start(mask[:], mask_d[:])
                nc.sync.dma_start(ones_sb[:], ones_d[:])
                nc.vector.tensor_copy(
                    vext[:, :, :, 64:65],
                    ones_sb[:, 0:64].rearrange("p (a b) -> p a b", a=NTT),
                )
                nc.scalar.dma_start(wv[:, 0:8, :], drview(wv_d, 0, 8, HH * D))
                nc.sync.dma_start(
                    woT[:, :, :],
                    woT_d[:, :].rearrange("(c p) t -> p c t", p=128))

                # ---------- phase 1a: q then k (all 8 PSUM banks) ----------
                with tc.tile_pool(name="ps_qk", bufs=4, space="PSUM") as ps_qk:
                    # PE p-state warmup on a zeroed tile while DMA lands
                    warm = xp.tile([128, 512], BF16)
                    nc.vector.memset(warm[:], 0.0)
                    # preload the Exp activation table while ACT is idle
                    scrap = xp.tile([1, 2], BF16)
                    nc.scalar.activation(scrap[:], warm[0:1, 0:2], Exp, scale=0.125)
                    psw = ps_qk.tile([128, T], F32, tag="qk", name="psw")
                    for i in range(7):
                        mm(psw[:, 0:512], warm[:, 0:128], warm[:],
                           start=True, stop=True)

                    for what, w, dst in (("q", wq, qT), ("k", wk, kT)):
                        ps = {}
                        for p in range(NP):
                            ps[p] = ps_qk.tile([128, T], F32, tag="qk",
                                               name=f"ps{what}{p}")
                        for c in range(NCT):
                            for p in range(NP):
                                wc = w[:, c, p * 128:(p + 1) * 128]
                                for half in range(2):
                                    s = slice(half * 512, (half + 1) * 512)
                                    mm(ps[p][:, s], wc, xT[:, c, s],
                                       start=(c == 0), stop=(c == NCT - 1))
                        for p in range(NP):
                            if p % 2 == 0:
                                nc.scalar.copy(dst[:, p, :], ps[p][:])
                            else:
                                nc.vector.tensor_copy(dst[:, p, :], ps[p][:])

                # ---------- phases 1b + 2 + 3 ----------
                with (
                    tc.tile_pool(name="attn", bufs=2 + 6 * (PRELUDE + 1) + 6) as attnp,
                    tc.tile_pool(name="smalls", bufs=3) as smalls,
                    tc.tile_pool(name="ps_s", bufs=3, space="PSUM") as ps_s,
                ):
                    ats = {}
                    oTs = {}

                    def s_tile(h, ti, lst):
                        p, po = h // 2, (h % 2) * 64
                        slots = SCORE_TILES[ti]
                        sps = ps_s.tile([128, T], F32, tag="s")
                        for si, (kb, q0, ln) in enumerate(slots):
                            col = si * 512
                            mm(sps[:, col:col + ln],
                               kT[po:po + 64, p, kb * 128:(kb + 1) * 128],
                               qT[po:po + 64, p, q0:q0 + ln],
                               start=True, stop=True)
                        at = attnp.tile([128, T], BF16, tag="at")
                        for c0, c1 in EXP_RANGES[ti]:
                            nc.scalar.activation(at[:, c0:c1], sps[:, c0:c1],
                                                 Exp, scale=0.125)
                        for mti, col in MASK_POS:
                            if mti == ti:
                                nc.gpsimd.tensor_mul(
                                    at[:, col:col + 128],
                                    at[:, col:col + 128],
                                    mask[:],
                                )
                        lst.append(at)

                    def stage_s(h):
                        lst = []
                        for ti in range(len(SCORE_TILES)):
                            s_tile(h, ti, lst)
                        ats[h] = lst

                    def a_half(h, half, oTh, lst):
                        # AV pieces whose oT bank == half, cols rebased
                        base = half * 512
                        for ti, slots in enumerate(SCORE_TILES):
                            at = lst[ti]
                            for si, (kb, q0, ln) in enumerate(slots):
                                col = si * 512
                                subs = []
                                if q0 < 512:
                                    subs.append((col, q0, min(ln, 512 - q0)))
                                    if q0 + ln > 512:
                                        subs.append((col + 512 - q0, 512,
                                                     q0 + ln - 512))
                                else:
                                    subs.append((col, q0, ln))
                                for scol, sq0, sln in subs:
                                    if (sq0 >= 512) != (half == 1):
                                        continue
                                    last = (kb == 3 and half == 0) or kb == 7
                                    mm(oTh[:, sq0 - base:sq0 - base + sln],
                                       vext[:, kb, h, :],
                                       at[:, scol:scol + sln],
                                       start=(kb == 0), stop=last,
                                       skip_group_check=True)

                    def n_half(h, half, oTh):
                        p, po = h // 2, (h % 2) * 64
                        base = half * 512
                        rec = smalls.tile([1, 512], F32, tag="rec")
                        nc.vector.reciprocal(rec[:], oTh[64:65, :])
                        bc = smalls.tile([64, 512], F32, tag="bc")
                        nc.gpsimd.partition_broadcast(bc[:], rec[:])
                        nc.vector.tensor_mul(
                            proj_in[po:po + 64, p, base:base + 512],
                            oTh[0:64, :], bc[:])

                    # -------- phase 1b: v interleaved with S(0..PRELUDE-1) --
                    with tc.tile_pool(name="ps_v", bufs=2, space="PSUM") as ps_v:
                        vjobs = []
                        for tt in range(NTT):
                            vjobs.append(("alloc", tt))
                            for c in range(NCT):
                                vjobs.append(("mm", tt, c))
                            vjobs.append(("copy", tt))
                        sjobs = [("s", h) for h in range(PRELUDE)]
                        # interleave: ~3 v-jobs per score stage
                        psvs = {}
                        vi = si_ = 0
                        while vi < len(vjobs) or si_ < len(sjobs):
                            for _ in range(30):
                                if vi >= len(vjobs):
                                    break
                                job = vjobs[vi]
                                vi += 1
                                if job[0] == "alloc":
                                    tt = job[1]
                                    psvw = ps_v.tile([128, 512], F32, tag="v",
                                                     name=f"psv{tt}")
                                    psvs[tt] = psvw
                                elif job[0] == "mm":
                                    _, tt, c = job
                                    mm(psvs[tt][:],
                                       xT[:, c, tt * 128:(tt + 1) * 128],
                                       wv[:, c, :],
                                       start=(c == 0), stop=(c == NCT - 1))
                                else:
                                    tt = job[1]
                                    src = psvs.pop(tt)[:].rearrange(
                                        "p (h d) -> p h d", h=HH)
                                    if tt % 2 == 0:
                                        nc.scalar.copy(vext[:, tt, :, 0:64], src)
                                    else:
                                        nc.vector.tensor_copy(
                                            vext[:, tt, :, 0:64], src)
                            if si_ < len(sjobs):
                                stage_s(sjobs[si_][1])
                                si_ += 1

                    # -------- phase 2 main: pipeline S/A/N over heads.  ps_s
                    # triple-buffered (6 banks); oT split into two 1-bank
                    # halves (bufs=2) normalized as soon as each half's AV
                    # accumulation completes, spreading N work through the
                    # period.
                    with tc.tile_pool(name="ps_o", bufs=2, space="PSUM") as ps_o:
                        for h in range(HH):
                            if h + PRELUDE < HH:
                                stage_s(h + PRELUDE)
                            alst = ats.pop(h)
                            oTa = ps_o.tile([65, 512], F32, tag="o", name="oTa")
                            a_half(h, 0, oTa, alst)
                            n_half(h, 0, oTa)
                            oTb = ps_o.tile([65, 512], F32, tag="o", name="oTb")
                            a_half(h, 1, oTb, alst)
                            n_half(h, 1, oTb)

            # ---------------- phase 3: partial output projection ----------
            with (
                tc.tile_pool(name="fin", bufs=4) as finp,
                tc.tile_pool(name="ps_f", bufs=2, space="PSUM") as ps_f,
            ):
                for m in range(T // 128):
                    psf = ps_f.tile([128, C], F32, tag="f")
                    for p in range(NP):
                        lhs = proj_in[:, p, m * 128:(m + 1) * 128]
                        mm(psf[:, 0:512], lhs, woT[:, p, 0:512],
                           start=(p == 0), stop=(p == NP - 1))
                        mm(psf[:, 512:1024], lhs, woT[:, p, 512:1024],
                           start=(p == 0), stop=(p == NP - 1))
                    fin = finp.tile([128, C], BF16, tag="fin")
                    if m % 2 == 0:
                        nc.scalar.copy(fin[:], psf[:])
                    else:
                        nc.vector.tensor_copy(fin[:], psf[:])
                    ring = nc.sync if m % 2 == 0 else nc.scalar
                    ring.dma_start(out_d[m * 128:(m + 1) * 128, :], fin[:])

    nc.compile()
    return nc


def get_nc():
    if "nc" not in _CACHE:
        _CACHE["nc"] = _build()
    return _CACHE["nc"]


def make_in_maps(x, Wq, Wk, Wv, Wo, bo):
    import ml_dtypes

    bf16 = ml_dtypes.bfloat16
    x = np.asarray(x, dtype=np.float32)
    Wq = np.asarray(Wq, np.float32)
    Wk = np.asarray(Wk, np.float32)
    Wv = np.asarray(Wv, np.float32)
    Wo = np.asarray(Wo, np.float32)
    k_ = np.arange(128)[:, None]
    i_ = np.arange(128)[None, :]
    mask = (k_ <= i_).astype(bf16)
    ones = np.ones((128, 128), bf16)
    xTs = [np.ascontiguousarray(x[b].T).astype(bf16) for b in range(B)]
    whalf = {}
    for hh in range(2):
        sl = slice(hh * HH, (hh + 1) * HH)
        whalf[hh] = {
            "wq": Wq[sl].transpose(1, 0, 2).reshape(C, HH * D).astype(bf16),
            "wk": Wk[sl].transpose(1, 0, 2).reshape(C, HH * D).astype(bf16),
            "wv": Wv[sl].transpose(1, 0, 2).reshape(C, HH * D).astype(bf16),
            "woT": np.ascontiguousarray(
                Wo[:, hh * HH * D:(hh + 1) * HH * D].T).astype(bf16),
        }
    in_maps = []
    for core in range(N_CORES):
        b, hh = core // 2, core % 2
        m = {"xT": xTs[b], "mask": mask, "ones": ones}
        m.update(whalf[hh])
        in_maps.append(m)
    return in_maps


def kernel(x, Wq, Wk, Wv, Wo, bo):
    from concourse.bass_utils import run_bass_kernel_spmd

    nc = get_nc()
    in_maps = make_in_maps(x, Wq, Wk, Wv, Wo, bo)
    res = run_bass_kernel_spmd(nc, in_maps, list(range(N_CORES)))
    _CACHE["last_result"] = res
    bo = np.asarray(bo, np.float32)
    out = np.empty((B, T, C), np.float32)
    for b in range(B):
        out[b] = (res.results[2 * b]["out"].astype(np.float32)
                  + res.results[2 * b + 1]["out"].astype(np.float32) + bo)
    return out


# revision 40
# speedup vs baseline: 1.6685x; 1.0016x over previous
"""Trainium2 Bass kernel for causal MHA (B=4,T=1024,C=1024,H=16,D=64).

Sharding: 8 cores = 4 batches x 2 head-halves (8 heads per core, full T).
Each core computes q/k/v projections for its 8 heads, causal attention over
all 1024 queries, and a PARTIAL output projection (its heads' rows of Wo).
The two cores of a batch produce partials the host sums (+bias) during
output assembly — the tensor-parallel "all-reduce" is free on host, so the
device program needs no collectives and no duplicated k/v work.

All operands are bf16 (inputs pre-converted on host); PSUM accumulation and
the softmax denominators stay f32.  bf16 halves DMA and SBUF and runs the
PE at 1 cycle/row for every piece size.

Per-core device program:
  phase 1a: qT[p] = Wq_p^T @ xT, kT[p] = Wk_p^T @ xT (PE, 8 PSUM banks).
  phase 1b: v = x @ Wv -> vext [keys, 65] (+ones col), INTERLEAVED with the
            first 3 heads' score stages so the ACT exp pipeline starts early.
  phase 2: per head h: scoresT slot-packed [keys, queries] PSUM tiles (one
           matmul per bank, only valid causal cells); exp via ACT
           (scale=1/8); diagonal 128x128 blocks masked via tril mul on Pool;
           AV accumulates oT[65,1024] f32 = [v|1]^T @ attnT (row 64 =
           sum-exp); normalize: DVE reciprocal -> gpsimd partition_broadcast
           -> DVE mul (casts to bf16).  Heads software-pipelined A/N/S.
  phase 3: partial out = concat-heads @ WoT_half (PE), copy, DMA out (bf16);
           host upcasts, sums core pairs, adds bias.
"""
import sys

sys.path.insert(0, "/opt/trn_rl_repo")
import numpy as np

B, T, C, H, D = 4, 1024, 1024, 16, 64
N_CORES = 8
HH = H // 2  # heads per core
NP = HH // 2  # head pairs per core (partition-stacked)
NCT = C // 128  # contraction tiles
NTT = T // 128  # key blocks
PRELUDE = 3  # heads whose S stage is interleaved with the v projection

# scores tiling per head: tiles of [128, 1024] PSUM (2 banks / 2 slots of
# 512).  Every slot gets EXACTLY ONE matmul whose output is contained in that
# bank (PE matmul output must not cross a PSUM bank boundary, and each bank
# holds a single start/stop accumulation group).  Slot entries: (kb, q0, ln)
# with tile cols [512*slot : 512*slot + ln) <-> queries [q0 : q0+ln).
# Full 512-slots are paired with partial slots so each tile's valid region
# is CONTIGUOUS [0 : 512+ln2) -> exactly one exp instruction per tile.
SCORE_TILES = [
    [(0, 0, 512), (1, 640, 384)],
    [(0, 512, 512), (2, 768, 256)],
    [(1, 128, 512), (3, 896, 128)],
    [(2, 256, 512), (5, 640, 384)],
    [(3, 384, 512), (6, 768, 256)],
    [(4, 512, 512), (7, 896, 128)],
]
# diagonal-mask positions: (tile_idx, tile_col) for each kb's diag block
MASK_POS = [(0, 0), (2, 0), (3, 0), (4, 0), (5, 0), (3, 512), (4, 512), (5, 512)]


def _av_pieces():
    """AV matmul pieces per oT bank, ordered by kb (PSUM accumulation order).
    Returns {half: [(kb, at_tile, at_col, oT_col, ln), ...]}."""
    per_half = {0: [], 1: []}
    for ti, slots in enumerate(SCORE_TILES):
        for si, (kb, q0, ln) in enumerate(slots):
            col = si * 512
            subs = []
            if q0 < 512:
                subs.append((col, q0, min(ln, 512 - q0)))
                if q0 + ln > 512:
                    subs.append((col + 512 - q0, 512, q0 + ln - 512))
            else:
                subs.append((col, q0, ln))
            for scol, sq0, sln in subs:
                half = int(sq0 >= 512)
                per_half[half].append((kb, ti, scol, sq0 - half * 512, sln))
    for half in per_half:
        per_half[half].sort(key=lambda x: (x[0], x[3]))
    return per_half


AV_PIECES = _av_pieces()

_CACHE = {}


def _build():
    import concourse.bacc as bacc
    import concourse.mybir as mybir
    import concourse.tile as tile
    from concourse import library_config

    F32 = mybir.dt.float32
    BF16 = mybir.dt.bfloat16
    Exp = mybir.ActivationFunctionType.Exp

    nc = bacc.Bacc("TRN2", target_bir_lowering=False, debug=False, num_devices=N_CORES)
    xT_d = nc.declare_dram_parameter("xT", [C, T], BF16, isOutput=False)
    wq_d = nc.declare_dram_parameter("wq", [C, HH * D], BF16, isOutput=False)
    wk_d = nc.declare_dram_parameter("wk", [C, HH * D], BF16, isOutput=False)
    wv_d = nc.declare_dram_parameter("wv", [C, HH * D], BF16, isOutput=False)
    woT_d = nc.declare_dram_parameter("woT", [HH * D, C], BF16, isOutput=False)
    mask_d = nc.declare_dram_parameter("mask", [128, 128], BF16, isOutput=False)
    ones_d = nc.declare_dram_parameter("ones", [128, 128], BF16, isOutput=False)
    out_d = nc.declare_dram_parameter("out", [T, C], BF16, isOutput=True)

    mm = nc.tensor.matmul

    with tile.TileContext(nc) as tc:
        with tc.tile_pool(name="keep", bufs=1) as keep:
            qT = keep.tile([128, NP, T], BF16)
            kT = keep.tile([128, NP, T], BF16)
            vext = keep.tile([128, NTT, HH, 65], BF16)
            mask = keep.tile([128, 128], BF16)
            ones_sb = keep.tile([128, 128], BF16)
            proj_in = keep.tile([128, NP, T], BF16)
            woT = keep.tile([128, NP, C], BF16)
            nc.gpsimd.load_library(library_config.proxy)

            with tc.tile_pool(name="xp", bufs=1) as xp:
                xT = xp.tile([128, NCT, T], BF16)
                wq = xp.tile([128, NCT, HH * D], BF16)
                wk = xp.tile([128, NCT, HH * D], BF16)
                wv = xp.tile([128, NCT, HH * D], BF16)
                # batched transfers (HWDGE generation is ~630ns per DMACopy
                # and serialized, so fewer/bigger transfers win); c0 split so
                # the first matmuls start ASAP.
                def drview(d, c0, c1, cols):
                    return d[c0 * 128:c1 * 128, 0:cols].rearrange(
                        "(c p) t -> p c t", p=128)

                nc.sync.dma_start(xT[:, 0, 0:512], xT_d[0:128, 0:512])
                nc.scalar.dma_start(wq[:, 0, :], wq_d[0:128, :])
                nc.sync.dma_start(xT[:, 0, 512:1024], xT_d[0:128, 512:1024])
                nc.sync.dma_start(xT[:, 1:4, :], drview(xT_d, 1, 4, T))
                nc.scalar.dma_start(wq[:, 1:4, :], drview(wq_d, 1, 4, HH * D))
                nc.scalar.dma_start(wq[:, 4:8, :], drview(wq_d, 4, 8, HH * D))
                nc.sync.dma_start(xT[:, 4:8, :], drview(xT_d, 4, 8, T))
                nc.scalar.dma_start(wk[:, 0:4, :], drview(wk_d, 0, 4, HH * D))
                nc.scalar.dma_start(wk[:, 4:8, :], drview(wk_d, 4, 8, HH * D))
                nc.sync.dma_start(mask[:], mask_d[:])
                nc.sync.dma_start(ones_sb[:], ones_d[:])
                nc.vector.tensor_copy(
                    vext[:, :, :, 64:65],
                    ones_sb[:, 0:64].rearrange("p (a b) -> p a b", a=NTT),
                )
                nc.scalar.dma_start(wv[:, 0:8, :], drview(wv_d, 0, 8, HH * D))
                nc.sync.dma_start(
                    woT[:, :, :],
                    woT_d[:, :].rearrange("(c p) t -> p c t", p=128))

                # ---------- phase 1a: q then k (all 8 PSUM banks) ----------
                # ---------- phases 1 + 2 ----------
                with (
                    tc.tile_pool(name="attn", bufs=2 + 6 * (PRELUDE + 1) + 6) as attnp,
                    tc.tile_pool(name="smalls", bufs=4) as smalls,
                ):
                    ats = {}
                    oTs = {}
                    ps_s = None  # bound after phase 1a (PSUM bank budget)

                    def s_tile(h, ti, lst):
                        p, po = h // 2, (h % 2) * 64
                        slots = SCORE_TILES[ti]
                        sps = ps_s.tile([128, T], F32, tag="s")
                        for si, (kb, q0, ln) in enumerate(slots):
                            col = si * 512
                            mm(sps[:, col:col + ln],
                               kT[po:po + 64, p, kb * 128:(kb + 1) * 128],
                               qT[po:po + 64, p, q0:q0 + ln],
                               start=True, stop=True)
                        at = attnp.tile([128, T], BF16, tag="at")
                        vl = 512 + slots[1][2]
                        nc.scalar.activation(at[:, 0:vl], sps[:, 0:vl],
                                             Exp, scale=0.125)
                        for mti, col in MASK_POS:
                            if mti == ti:
                                nc.gpsimd.tensor_mul(
                                    at[:, col:col + 128],
                                    at[:, col:col + 128],
                                    mask[:],
                                )
                        lst.append(at)

                    def stage_s(h):
                        lst = []
                        for ti in range(len(SCORE_TILES)):
                            s_tile(h, ti, lst)
                        ats[h] = lst

                    def a_half(h, half, oTh, lst):
                        pieces = AV_PIECES[half]
                        for i, (kb, ti, scol, ocol, sln) in enumerate(pieces):
                            mm(oTh[:, ocol:ocol + sln],
                               vext[:, kb, h, :],
                               lst[ti][:, scol:scol + sln],
                               start=(i == 0), stop=(i == len(pieces) - 1),
                               skip_group_check=True)

                    def n_half(h, half, oTh):
                        p, po = h // 2, (h % 2) * 64
                        base = half * 512
                        rec = smalls.tile([1, 512], F32, tag="rec")
                        nc.vector.reciprocal(rec[:], oTh[64:65, :])
                        bc = smalls.tile([64, 512], F32, tag="bc")
                        nc.gpsimd.partition_broadcast(bc[:], rec[:])
                        nc.vector.tensor_mul(
                            proj_in[po:po + 64, p, base:base + 512],
                            oTh[0:64, :], bc[:])

                    # -------- phase 1a: q then k, all 4 pairs c-major -------
                    with tc.tile_pool(name="ps_qk", bufs=4, space="PSUM") as ps_qk:
                        # PE p-state warmup on a zeroed tile while DMA lands
                        warm = xp.tile([128, 512], BF16)
                        nc.vector.memset(warm[:], 0.0)
                        # preload the Exp activation table while ACT is idle
                        scrap = xp.tile([1, 2], BF16)
                        nc.scalar.activation(scrap[:], warm[0:1, 0:2], Exp,
                                             scale=0.125)
                        psw = ps_qk.tile([128, T], F32, tag="qk", name="psw")
                        for i in range(7):
                            mm(psw[:, 0:512], warm[:, 0:128], warm[:],
                               start=True, stop=True)

                        for what, w, dst in (("q", wq, qT), ("k", wk, kT)):
                            ps = {}
                            for p in range(NP):
                                ps[p] = ps_qk.tile([128, T], F32, tag="qk",
                                                   name=f"ps{what}{p}")
                            for c in range(NCT):
                                for p in range(NP):
                                    wc = w[:, c, p * 128:(p + 1) * 128]
                                    for half in range(2):
                                        s = slice(half * 512, (half + 1) * 512)
                                        mm(ps[p][:, s], wc, xT[:, c, s],
                                           start=(c == 0), stop=(c == NCT - 1))
                            for p in range(NP):
                                if p % 2 == 0:
                                    nc.scalar.copy(dst[:, p, :], ps[p][:])
                                else:
                                    nc.vector.tensor_copy(dst[:, p, :], ps[p][:])

                    # -------- phase 1b: v interleaved with S(0..PRELUDE-1) --
                    with tc.tile_pool(name="ps_s", bufs=3, space="PSUM") as _pss:
                        ps_s = _pss

                        with tc.tile_pool(name="ps_v", bufs=2, space="PSUM") as ps_v:
                            def v_tt(tt):
                                psvw = ps_v.tile([128, 512], F32, tag="v",
                                                 name=f"psv{tt}")
                                for c in range(NCT):
                                    mm(psvw[:],
                                       xT[:, c, tt * 128:(tt + 1) * 128],
                                       wv[:, c, :],
                                       start=(c == 0), stop=(c == NCT - 1))
                                src = psvw[:].rearrange("p (h d) -> p h d", h=HH)
                                if tt % 2 == 0:
                                    nc.scalar.copy(vext[:, tt, :, 0:64], src)
                                else:
                                    nc.vector.tensor_copy(vext[:, tt, :, 0:64],
                                                          src)

                            for tt in range(NTT):
                                v_tt(tt)
                                if tt % 3 == 1 and tt // 3 < PRELUDE:
                                    stage_s(tt // 3)

                        # ---- phase 2 main: pipeline S/A/N over heads ----
                        with tc.tile_pool(name="ps_o", bufs=2, space="PSUM") as ps_o:
                            for h in range(HH):
                                if h + PRELUDE < HH:
                                    stage_s(h + PRELUDE)
                                alst = ats.pop(h)
                                oTa = ps_o.tile([65, 512], F32, tag="o",
                                                name="oTa")
                                a_half(h, 0, oTa, alst)
                                n_half(h, 0, oTa)
                                oTb = ps_o.tile([65, 512], F32, tag="o",
                                                name="oTb")
                                a_half(h, 1, oTb, alst)
                                n_half(h, 1, oTb)

            # ---------------- phase 3: partial output projection ----------
            with (
                tc.tile_pool(name="fin", bufs=4) as finp,
                tc.tile_pool(name="ps_f", bufs=2, space="PSUM") as ps_f,
            ):
                for m in range(T // 128):
                    psf = ps_f.tile([128, C], F32, tag="f")
                    for p in range(NP):
                        lhs = proj_in[:, p, m * 128:(m + 1) * 128]
                        mm(psf[:, 0:512], lhs, woT[:, p, 0:512],
                           start=(p == 0), stop=(p == NP - 1))
                        mm(psf[:, 512:1024], lhs, woT[:, p, 512:1024],
                           start=(p == 0), stop=(p == NP - 1))
                    fin = finp.tile([128, C], BF16, tag="fin")
                    if m % 2 == 0:
                        nc.scalar.copy(fin[:], psf[:])
                    else:
                        nc.vector.tensor_copy(fin[:], psf[:])
                    ring = nc.sync if m % 2 == 0 else nc.scalar
                    ring.dma_start(out_d[m * 128:(m + 1) * 128, :], fin[:])

    nc.compile()
    return nc


def get_nc():
    if "nc" not in _CACHE:
        _CACHE["nc"] = _build()
    return _CACHE["nc"]


def make_in_maps(x, Wq, Wk, Wv, Wo, bo):
    import ml_dtypes

    bf16 = ml_dtypes.bfloat16
    x = np.asarray(x, dtype=np.float32)
    Wq = np.asarray(Wq, np.float32)
    Wk = np.asarray(Wk, np.float32)
    Wv = np.asarray(Wv, np.float32)
    Wo = np.asarray(Wo, np.float32)
    k_ = np.arange(128)[:, None]
    i_ = np.arange(128)[None, :]
    mask = (k_ <= i_).astype(bf16)
    ones = np.ones((128, 128), bf16)
    xTs = [np.ascontiguousarray(x[b].T).astype(bf16) for b in range(B)]
    whalf = {}
    for hh in range(2):
        sl = slice(hh * HH, (hh + 1) * HH)
        whalf[hh] = {
            "wq": Wq[sl].transpose(1, 0, 2).reshape(C, HH * D).astype(bf16),
            "wk": Wk[sl].transpose(1, 0, 2).reshape(C, HH * D).astype(bf16),
            "wv": Wv[sl].transpose(1, 0, 2).reshape(C, HH * D).astype(bf16),
            "woT": np.ascontiguousarray(
                Wo[:, hh * HH * D:(hh + 1) * HH * D].T).astype(bf16),
        }
    in_maps = []
    for core in range(N_CORES):
        b, hh = core // 2, core % 2
        m = {"xT": xTs[b], "mask": mask, "ones": ones}
        m.update(whalf[hh])
        in_maps.append(m)
    return in_maps


def kernel(x, Wq, Wk, Wv, Wo, bo):
    from concourse.bass_utils import run_bass_kernel_spmd

    nc = get_nc()
    in_maps = make_in_maps(x, Wq, Wk, Wv, Wo, bo)
    res = run_bass_kernel_spmd(nc, in_maps, list(range(N_CORES)))
    _CACHE["last_result"] = res
    bo = np.asarray(bo, np.float32)
    out = np.empty((B, T, C), np.float32)
    for b in range(B):
        out[b] = (res.results[2 * b]["out"].astype(np.float32)
                  + res.results[2 * b + 1]["out"].astype(np.float32) + bo)
    return out
